# revision 3
# baseline (speedup 1.0000x reference)
"""PointNet segment_max kernel for 8 TRN2 NeuronCores.

Strategy:
- Host composes consecutive linear layers (no ReLU between them):
    enc2@in1, in2@max1, out3@fc@dec1, and splits the concat([z,gc])@Wo1
    matmul into a per-point part (8->128 from h8) and a per-graph part
    (gc @ Wo1[16:], only 16 distinct rows -> tiny table built on device
    after the segment max).
- Points are sharded per (core, graph): each graph's points are split
  evenly over the 8 cores and padded (with duplicated same-graph points)
  to a multiple of 512 so that every 512-point chunk belongs to exactly
  one statically-known graph. Segment max then needs no masks/gathers.
- Device (SPMD, one launch): feature-major bf16 activations,
  pass1: x -> h1 -> h8(saved to DRAM) -> hm1 -> hm2 -> mm5 (256 feats)
         -> per-chunk reduce_max -> per-graph local max
  AllReduce(max) of the [256,16] table across cores,
  col table = (gmax + bm3) @ gcW + c6  (per-graph bias column),
  pass2: h8 -> u(+col[g]) -> v -> s -> y.
"""
import math
import numpy as np
import ml_dtypes

import concourse.bacc as bacc
import concourse.tile as tile
import concourse.mybir as mybir
from concourse.bass_utils import run_bass_kernel_spmd

BF16 = mybir.dt.bfloat16
F32 = mybir.dt.float32
NPBF16 = ml_dtypes.bfloat16

NCORES = 8
G = 16
CH = 512

_prog_cache = {}
_last = {}


def _f(a):
    return np.asarray(a, dtype=np.float64)


def _compose_weights(enc_params, in_params, max_params, out_params, fc_W, fc_b,
                     dec_params):
    (We1, be1), (We2, be2) = [(_f(w), _f(b)) for (w, b) in enc_params]
    (Wi1, bi1), (Wi2, bi2) = [(_f(w), _f(b)) for (w, b) in in_params]
    (Wm1, bm1), (Wm2, bm2), (Wm3, bm3) = [(_f(w), _f(b)) for (w, b) in max_params]
    (Wo1, bo1), (Wo2, bo2), (Wo3, bo3) = [(_f(w), _f(b)) for (w, b) in out_params]
    fc_W, fc_b = _f(fc_W), _f(fc_b)
    (Wd1, bd1), (Wd2, bd2) = [(_f(w), _f(b)) for (w, b) in dec_params]

    W = {}
    W["A1"] = We1                      # [7,64]
    W["c1"] = be1
    W["A2"] = We2 @ Wi1                # [64,8]
    W["c2"] = be2 @ Wi1 + bi1
    W["A3"] = Wi2 @ Wm1                # [8,32]
    W["c3"] = bi2 @ Wm1 + bm1
    W["A4"] = Wm2                      # [32,64]
    W["c4"] = bm2
    W["A5"] = Wm3                      # [64,256]
    W["c5"] = bm3                      # [256] applied post-max
    W["A6"] = Wi2 @ Wo1[:16]           # [8,128]
    W["c6"] = bi2 @ Wo1[:16] + bo1     # [128]
    W["gcW"] = Wo1[16:272]             # [256,128]
    W["A7"] = Wo2                      # [128,64]
    W["c7"] = bo2
    W["A8"] = Wo3 @ fc_W @ Wd1         # [64,64]
    W["c8"] = (bo3 @ fc_W + fc_b) @ Wd1 + bd1
    W["A9"] = Wd2                      # [64,4]
    W["c9"] = bd2
    return {k: np.asarray(v, dtype=np.float32) for k, v in W.items()}


# weight blob layout: (name, rows, cols, col_offset) in bf16 elements
_WSPEC = [
    ("A1", 7, 64, 0),
    ("A2", 64, 8, 64),
    ("A3", 8, 32, 72),
    ("A4", 32, 64, 104),
    ("A5", 64, 256, 168),
    ("A6", 8, 128, 424),
    ("A7", 128, 64, 552),
    ("A8", 64, 64, 616),
    ("A9", 64, 4, 680),
    ("gcWa", 128, 128, 684),
    ("gcWb", 128, 128, 812),
]
_WBW = 940
# bias blob: (name, rows, col)
_BSPEC = [
    ("c1", 64, 0), ("c2", 8, 1), ("c3", 32, 2), ("c4", 64, 3),
    ("c5a", 128, 4), ("c5b", 128, 5), ("c6", 128, 6), ("c7", 64, 7),
    ("c8", 64, 8), ("c9", 4, 9),
]
_NB = 10


def _pack_blobs(W):
    parts = dict(W)
    parts["gcWa"] = W["gcW"][0:128]
    parts["gcWb"] = W["gcW"][128:256]
    parts["c5a"] = W["c5"][0:128]
    parts["c5b"] = W["c5"][128:256]
    wb = np.zeros((128, _WBW), dtype=NPBF16)
    for name, rows, cols, off in _WSPEC:
        wb[0:rows, off:off + cols] = parts[name].astype(NPBF16)
    bb = np.zeros((128, _NB), dtype=np.float32)
    for name, rows, col in _BSPEC:
        bb[0:rows, col] = parts[name]
    return wb, bb


def _build_program(P, with_collective=True):
    NCH = P // CH
    R = NCH // G
    nc = bacc.Bacc("TRN2", target_bir_lowering=False, debug=False,
                   num_devices=NCORES)
    x_d = nc.dram_tensor("x_t", [7, P], BF16, kind="ExternalInput").ap()
    wb_d = nc.dram_tensor("wblob", [128, _WBW], BF16, kind="ExternalInput").ap()
    bb_d = nc.dram_tensor("bblob", [128, _NB], F32, kind="ExternalInput").ap()
    y_d = nc.dram_tensor("y_t", [4, P], F32, kind="ExternalOutput").ap()
    h8_d = nc.dram_tensor("h8buf", [8, P], BF16).ap()

    RELU = mybir.ActivationFunctionType.Relu
    ADD = mybir.AluOpType.add
    MAX = mybir.AluOpType.max
    AXX = mybir.AxisListType.X

    with tile.TileContext(nc) as tc:
        with tc.tile_pool(name="const", bufs=1) as cpool, \
             tc.tile_pool(name="acts", bufs=1) as apool, \
             tc.tile_pool(name="ps", bufs=8, space="PSUM") as pspool, \
             tc.tile_pool(name="dram", bufs=1, space="DRAM") as dpool:

            wsb = cpool.tile([128, _WBW], BF16, tag="wsb")
            bsb = cpool.tile([128, _NB], F32, tag="bsb")
            nc.sync.dma_start(wsb[:], wb_d[:])
            nc.sync.dma_start(bsb[:], bb_d[:])
            wv = {}
            for name, rows, cols, off in _WSPEC:
                wv[name] = wsb[0:rows, off:off + cols]
            bv = {}
            for name, rows, col in _BSPEC:
                bv[name] = bsb[0:rows, col:col + 1]

            gmax_a = cpool.tile([128, G], F32, tag="gmax_a")
            gmax_b = cpool.tile([128, G], F32, tag="gmax_b")

            # ---------------- pass 1 ----------------
            cm_a = cm_b = None
            for i in range(NCH):
                g = i // R
                r = i % R
                if r == 0:
                    cm_a = apool.tile([128, R], F32, tag="cm_a")
                    cm_b = apool.tile([128, R], F32, tag="cm_b")
                sl = slice(i * CH, (i + 1) * CH)
                x_sb = apool.tile([7, CH], BF16, tag="x")
                nc.sync.dma_start(x_sb[:], x_d[:, sl])

                ps1 = pspool.tile([64, CH], F32, tag="ps")
                nc.tensor.matmul(ps1[:], wv["A1"], x_sb[:], start=True, stop=True)
                h1 = apool.tile([64, CH], BF16, tag="h1")
                nc.scalar.activation(h1[:], ps1[:], RELU, bias=bv["c1"])

                ps2 = pspool.tile([8, CH], F32, tag="ps")
                nc.tensor.matmul(ps2[:], wv["A2"], h1[:], start=True, stop=True)
                h8 = apool.tile([8, CH], BF16, tag="h8")
                nc.vector.tensor_scalar(h8[:], ps2[:], bv["c2"], 0.0,
                                        op0=ADD, op1=MAX)
                nc.sync.dma_start(h8_d[:, sl], h8[:])

                ps3 = pspool.tile([32, CH], F32, tag="ps")
                nc.tensor.matmul(ps3[:], wv["A3"], h8[:], start=True, stop=True)
                hm1 = apool.tile([32, CH], BF16, tag="hm1")
                nc.scalar.activation(hm1[:], ps3[:], RELU, bias=bv["c3"])

                ps4 = pspool.tile([64, CH], F32, tag="ps")
                nc.tensor.matmul(ps4[:], wv["A4"], hm1[:], start=True, stop=True)
                hm2 = apool.tile([64, CH], BF16, tag="hm2")
                nc.scalar.activation(hm2[:], ps4[:], RELU, bias=bv["c4"])

                ps5a = pspool.tile([128, CH], F32, tag="ps")
                nc.tensor.matmul(ps5a[:], wv["A5"][:, 0:128], hm2[:],
                                 start=True, stop=True)
                nc.vector.reduce_max(cm_a[:, r:r + 1], ps5a[:], axis=AXX)
                ps5b = pspool.tile([128, CH], F32, tag="ps")
                nc.tensor.matmul(ps5b[:], wv["A5"][:, 128:256], hm2[:],
                                 start=True, stop=True)
                nc.vector.reduce_max(cm_b[:, r:r + 1], ps5b[:], axis=AXX)
                if r == R - 1:
                    nc.vector.reduce_max(gmax_a[:, g:g + 1], cm_a[:], axis=AXX)
                    nc.vector.reduce_max(gmax_b[:, g:g + 1], cm_b[:], axis=AXX)

            # ---------------- allreduce max ----------------
            cc_in = dpool.tile([256, G], F32)
            cc_out = dpool.tile([256, G], F32)
            nc.sync.dma_start(cc_in[0:128, :], gmax_a[:])
            nc.sync.dma_start(cc_in[128:256, :], gmax_b[:])
            if with_collective:
                nc.gpsimd.collective_compute(
                    "AllReduce", MAX,
                    replica_groups=[list(range(NCORES))],
                    ins=[cc_in.opt()], outs=[cc_out.opt()],
                )
            else:
                nc.sync.dma_start(cc_out[:], cc_in[:])
            gm2a = cpool.tile([128, G], F32, tag="gm2a")
            gm2b = cpool.tile([128, G], F32, tag="gm2b")
            nc.sync.dma_start(gm2a[:], cc_out[0:128, :])
            nc.sync.dma_start(gm2b[:], cc_out[128:256, :])

            # col table: (gmax + c5) @ gcW + c6   -> [128, G] f32
            gmb_a = cpool.tile([128, G], BF16, tag="gmb_a")
            gmb_b = cpool.tile([128, G], BF16, tag="gmb_b")
            nc.vector.tensor_scalar(gmb_a[:], gm2a[:], bv["c5a"], None, op0=ADD)
            nc.vector.tensor_scalar(gmb_b[:], gm2b[:], bv["c5b"], None, op0=ADD)
            colps = pspool.tile([128, G], F32, tag="ps")
            nc.tensor.matmul(colps[:], wv["gcWa"], gmb_a[:], start=True, stop=False)
            nc.tensor.matmul(colps[:], wv["gcWb"], gmb_b[:], start=False, stop=True)
            col_sb = cpool.tile([128, G], F32, tag="col")
            nc.vector.tensor_scalar(col_sb[:], colps[:], bv["c6"], None, op0=ADD)

            # ---------------- pass 2 ----------------
            for i in range(NCH):
                g = i // R
                sl = slice(i * CH, (i + 1) * CH)
                h8c = apool.tile([8, CH], BF16, tag="h8c")
                nc.sync.dma_start(h8c[:], h8_d[:, sl])

                ps6 = pspool.tile([128, CH], F32, tag="ps")
                nc.tensor.matmul(ps6[:], wv["A6"], h8c[:], start=True, stop=True)
                u = apool.tile([128, CH], BF16, tag="u")
                nc.scalar.activation(u[:], ps6[:], RELU, bias=col_sb[:, g:g + 1])

                ps7 = pspool.tile([64, CH], F32, tag="ps")
                nc.tensor.matmul(ps7[:], wv["A7"], u[:], start=True, stop=True)
                v = apool.tile([64, CH], BF16, tag="v")
                nc.scalar.activation(v[:], ps7[:], RELU, bias=bv["c7"])

                ps8 = pspool.tile([64, CH], F32, tag="ps")
                nc.tensor.matmul(ps8[:], wv["A8"], v[:], start=True, stop=True)
                s = apool.tile([64, CH], BF16, tag="s")
                nc.vector.tensor_scalar(s[:], ps8[:], bv["c8"], 0.0,
                                        op0=ADD, op1=MAX)

                ps9 = pspool.tile([4, CH], F32, tag="ps")
                nc.tensor.matmul(ps9[:], wv["A9"], s[:], start=True, stop=True)
                y_sb = apool.tile([4, CH], F32, tag="y")
                nc.vector.tensor_scalar(y_sb[:], ps9[:], bv["c9"], None, op0=ADD)
                nc.sync.dma_start(y_d[:, sl], y_sb[:])

    nc.compile()
    return nc


def _layout(batch, n):
    """Per-core slot -> original point index (or n for dummy), plus P."""
    counts = np.bincount(batch, minlength=G).astype(np.int64)
    gstart = np.zeros(G + 1, dtype=np.int64)
    np.cumsum(counts, out=gstart[1:])
    # share sizes per (core, graph)
    base = counts // NCORES
    rem = counts % NCORES
    share = np.zeros((NCORES, G), dtype=np.int64)
    for c in range(NCORES):
        share[c] = base + (c < rem)
    R = max(1, int(math.ceil(share.max() / CH)))
    P = G * R * CH
    idx = np.full((NCORES, P), n, dtype=np.int64)
    for g in range(G):
        off = gstart[g]
        for c in range(NCORES):
            cnt = int(share[c, g])
            s0 = g * R * CH
            if cnt > 0:
                idx[c, s0:s0 + cnt] = np.arange(off, off + cnt)
                idx[c, s0 + cnt:(g + 1) * R * CH] = off  # pad: first pt of share
            elif counts[g] > 0:
                idx[c, s0:(g + 1) * R * CH] = gstart[g]  # any point of graph g
            off += cnt
    return idx, P


def kernel(x, batch, enc_params, in_params, max_params, out_params, fc_W, fc_b,
           dec_params):
    x = np.asarray(x, dtype=np.float32)
    batch = np.asarray(batch).astype(np.int64)
    n = x.shape[0]

    W = _compose_weights(enc_params, in_params, max_params, out_params,
                         fc_W, fc_b, dec_params)
    wb, bb = _pack_blobs(W)
    idx, P = _layout(batch, n)

    if P not in _prog_cache:
        _prog_cache[P] = _build_program(P)
    nc = _prog_cache[P]

    x_ext = np.vstack([x, np.zeros((1, 7), dtype=np.float32)])
    in_maps = []
    for c in range(NCORES):
        x_t = np.ascontiguousarray(x_ext[idx[c]].T).astype(NPBF16)
        in_maps.append({"x_t": x_t, "wblob": wb, "bblob": bb})

    res = run_bass_kernel_spmd(nc, in_maps, list(range(NCORES)))

    _last.clear()
    _last.update(nc=nc, in_maps=in_maps, res=res)

    y_full = np.zeros((n + 1, 4), dtype=np.float32)
    for c in range(NCORES):
        y_full[idx[c]] = res.results[c]["y_t"].T
    return y_full[:n]
